# revision 12
# baseline (speedup 1.0000x reference)
"""CharEmbeddingCNN Trainium2 kernel.

Reference computation (per word of L=20 chars):
    xe = emb[x]                       # [L, 256] -> treated as [256, L]
    y_k = conv1d_valid(xe, w_k) + b_k # k in (3,4,5), 256 -> 256 channels
    out = relu(max over all (k, t) of y_k[:, t]) * (len != 0)

Strategy (data-parallel over 8 NeuronCores, 1024 words each):
  - Host packs weights as bf16 lhsT tiles [i, (k,dk), o] and emb as a bf16
    DRAM table.
  - On device, `dma_gather(transpose=True)` gathers the embedding rows for
    a block of words directly into [128 part = ch%128, ch//128, pos] layout.
  - Conv = PSUM-accumulated matmuls over (i_chunk, dk) with shifted access
    patterns into the gathered block; word-groups of 28 give matmul free
    dims of 504/476/448 (= one PSUM bank).
  - Segment max over t via strided DVE reduce_max into per-k accumulators,
    then bias add + max-combine across k, PE transpose to [word, ch], and
    a fused relu*mask on ScalarE on the way out.
"""

import numpy as np
import ml_dtypes
from contextlib import ExitStack

import concourse.bass as bass
import concourse.bacc as bacc
import concourse.tile as tile
from concourse import masks, mybir
from concourse.bass_utils import run_bass_kernel_spmd

F32 = mybir.dt.float32
BF16 = mybir.dt.bfloat16
I16 = mybir.dt.int16

B, S, L = 64, 128, 20
EMB = 256
KS = (3, 4, 5)
NCORES = 8
W = (B * S) // NCORES          # words per core (1024)
GROUP = 24                     # words per matmul group (24*18=432 <= 512)
# Gather block: num_idxs = BLOCK*20 must be %128 == 0 (so BLOCK % 32 == 0)
# and <= ~2032 (SWDGE rx descriptor ring: num_idxs/8 + 2 <= 256 descs/lane).
BLOCK = 96
NKDK = sum(KS)                 # 12 packed (k, dk) weight slices


def _plan(words):
    """Split `words` into gather blocks (multiples of 32 words so that
    num_idxs % 128 == 0) and each block into matmul groups of <= GROUP."""
    blocks = []
    done = 0
    while done < words:
        bw = min(BLOCK, words - done)
        assert (bw * L) % 128 == 0, (words, bw)
        groups = []
        g = 0
        while g < bw:
            gw = min(GROUP, bw - g)
            groups.append((g, gw))
            g += gw
        blocks.append((done, bw, groups))
        done += bw
    return blocks


def _kdk_off(ki, dk):
    return sum(KS[:ki]) + dk


def build_bass(words=W):
    """Build the per-core Bass module. Returns (nc, input names)."""
    assert words % 128 == 0
    nwb = words // 128          # output word-blocks of 128 (transpose granularity)
    blocks = _plan(words)

    nc = bacc.Bacc(
        "TRN2",
        target_bir_lowering=False,
        debug=False,
        enable_asserts=False,
    )

    xi_cols = words * L // 16
    xi_d = nc.dram_tensor("xi", [128, xi_cols], I16, kind="ExternalInput").ap()
    emb_d = nc.dram_tensor("emb", [EMB, EMB], BF16, kind="ExternalInput").ap()
    wpk_d = nc.dram_tensor("wpk", [EMB, NKDK, EMB], BF16, kind="ExternalInput").ap()
    bias_d = nc.dram_tensor("bias", [128, 6], F32, kind="ExternalInput").ap()
    mask_d = nc.dram_tensor("maskp", [128, nwb], F32, kind="ExternalInput").ap()
    out_d = nc.dram_tensor("out", [words, EMB], F32, kind="ExternalOutput").ap()

    with tile.TileContext(nc) as tc, ExitStack() as ctx:
        const_pool = ctx.enter_context(tc.tile_pool(name="const", bufs=1))
        xe_pool = ctx.enter_context(tc.tile_pool(name="xe", bufs=2))
        psum_pool = ctx.enter_context(tc.tile_pool(name="ps", bufs=2, space="PSUM"))
        psum_t_pool = ctx.enter_context(tc.tile_pool(name="pst", bufs=2, space="PSUM"))
        m_pool = ctx.enter_context(tc.tile_pool(name="m", bufs=1))
        tmp_pool = ctx.enter_context(tc.tile_pool(name="tmp", bufs=2))
        out_pool = ctx.enter_context(tc.tile_pool(name="outp", bufs=3))

        ident = const_pool.tile([128, 128], F32)
        masks.make_identity(nc, ident[:])

        xi_t = const_pool.tile([128, xi_cols], I16)
        nc.sync.dma_start(xi_t[:], xi_d[:])
        wt = []
        for ic in range(2):
            t = const_pool.tile([128, NKDK, EMB], BF16, tag=f"wt{ic}")
            nc.sync.dma_start(t[:], wpk_d[ic * 128:(ic + 1) * 128, :, :])
            wt.append(t)
        bias_t = const_pool.tile([128, 6], F32)
        nc.sync.dma_start(bias_t[:], bias_d[:])
        mask_t = const_pool.tile([128, nwb], F32)
        nc.sync.dma_start(mask_t[:], mask_d[:])

        # per-(k, o_chunk) running max accumulators over all words
        M = {}
        for ki in range(3):
            for oc in range(2):
                M[(ki, oc)] = m_pool.tile(
                    [128, words], F32, tag=f"m{ki}{oc}", name=f"m{ki}{oc}")

        for (w0, bw, groups) in blocks:
            nidx = bw * L
            xe = xe_pool.tile([128, 2, nidx], BF16, tag="xe")
            nc.gpsimd.dma_gather(
                xe[:], emb_d[:], xi_t[:, w0 * L // 16: (w0 + bw) * L // 16],
                nidx, nidx, EMB, transpose=True, single_packet=False,
            )
            # [128, ch_chunk, word, t] views of the gathered block
            xv = [xe[:, ic, :].rearrange("p (w t) -> p w t", t=L) for ic in range(2)]
            for (g0, gw) in groups:
                for oc in range(2):
                    for ki, k in enumerate(KS):
                        lk = L - k + 1
                        ps = psum_pool.tile([128, gw, lk], F32, tag=f"ps{ki}")
                        n = 2 * k
                        i = 0
                        for ic in range(2):
                            for dk in range(k):
                                nc.tensor.matmul(
                                    ps[:],
                                    wt[ic][:, _kdk_off(ki, dk),
                                           oc * 128:(oc + 1) * 128],
                                    xv[ic][:, g0:g0 + gw, dk:dk + lk],
                                    start=(i == 0),
                                    stop=(i == n - 1),
                                )
                                i += 1
                        c0 = w0 + g0
                        nc.vector.reduce_max(
                            M[(ki, oc)][:, c0:c0 + gw], ps[:],
                            axis=mybir.AxisListType.X,
                        )

        for oc in range(2):
            cmb = tmp_pool.tile([128, words], F32, tag="cmb")
            t4 = tmp_pool.tile([128, words], F32, tag="t4")
            nc.vector.tensor_scalar_add(cmb[:], M[(0, oc)][:], bias_t[:, 3 * oc:3 * oc + 1])
            nc.vector.tensor_scalar_add(t4[:], M[(1, oc)][:], bias_t[:, 3 * oc + 1:3 * oc + 2])
            nc.vector.tensor_max(cmb[:], cmb[:], t4[:])
            nc.vector.tensor_scalar_add(t4[:], M[(2, oc)][:], bias_t[:, 3 * oc + 2:3 * oc + 3])
            nc.vector.tensor_max(cmb[:], cmb[:], t4[:])
            for wb in range(nwb):
                pst = psum_t_pool.tile([128, 128], F32, tag="pst")
                nc.tensor.transpose(pst[:], cmb[:, wb * 128:(wb + 1) * 128], ident[:])
                ot = out_pool.tile([128, 128], F32, tag="ot")
                nc.scalar.activation(
                    ot[:], pst[:], mybir.ActivationFunctionType.Relu,
                    scale=mask_t[:, wb:wb + 1],
                )
                nc.sync.dma_start(
                    out_d[wb * 128:(wb + 1) * 128, oc * 128:(oc + 1) * 128], ot[:],
                )
    nc.compile()
    return nc


def prep_shared(emb, w3, w4, w5, b3, b4, b5):
    emb_bf = np.ascontiguousarray(emb.astype(ml_dtypes.bfloat16))
    wpk = np.empty((EMB, NKDK, EMB), dtype=ml_dtypes.bfloat16)
    for ki, w in enumerate((w3, w4, w5)):
        k = KS[ki]
        for dk in range(k):
            # wpk[i, off, o] = w[o, i, dk]
            wpk[:, _kdk_off(ki, dk), :] = w[:, :, dk].T.astype(ml_dtypes.bfloat16)
    bias = np.empty((128, 6), dtype=np.float32)
    for oc in range(2):
        for ki, b in enumerate((b3, b4, b5)):
            bias[:, 3 * oc + ki] = b[oc * 128:(oc + 1) * 128]
    return emb_bf, wpk, bias


def prep_core(xf, lensf, words=W):
    """Per-core index + mask packing. xf: [words, L] int32, lensf: [words]."""
    xi = xf.reshape(-1).astype(np.int16)               # words * L
    # dma_gather index layout: idx j -> partition j % 16, column j // 16,
    # replicated across the 8 gpsimd cores (16-partition stripes).
    cols = xi.reshape(-1, 16).T                        # [16, words*L/16]
    xi_t = np.ascontiguousarray(np.tile(cols, (8, 1)))  # [128, cols]
    nwb = words // 128
    maskp = (lensf.reshape(nwb, 128).T != 0).astype(np.float32)
    maskp = np.ascontiguousarray(maskp)                # [128, nwb]
    return xi_t, maskp


_CACHE = {}


def _get_nc(words=W):
    if words not in _CACHE:
        _CACHE[words] = build_bass(words)
    return _CACHE[words]


def run(x, lens, emb, w3, b3, w4, b4, w5, b5, trace=False, **spmd_kwargs):
    x = np.asarray(x)
    lens = np.asarray(lens)
    emb = np.asarray(emb, dtype=np.float32)
    nc = _get_nc()
    emb_bf, wpk, bias = prep_shared(
        np.asarray(emb), np.asarray(w3), np.asarray(w4), np.asarray(w5),
        np.asarray(b3), np.asarray(b4), np.asarray(b5))
    xf = x.reshape(B * S, L)
    lensf = lens.reshape(B * S)
    in_maps = []
    for c in range(NCORES):
        sl = slice(c * W, (c + 1) * W)
        xi_t, maskp = prep_core(xf[sl], lensf[sl])
        in_maps.append({
            "xi": xi_t, "emb": emb_bf, "wpk": wpk, "bias": bias, "maskp": maskp,
        })
    res = run_bass_kernel_spmd(
        nc, in_maps, core_ids=list(range(NCORES)), trace=trace, **spmd_kwargs)
    out = np.concatenate([r["out"] for r in res.results], axis=0)
    return np.ascontiguousarray(out.reshape(B, S, EMB).astype(np.float32)), res


def kernel(x, lens, emb, w3, b3, w4, b4, w5, b5, **unused):
    out, _ = run(x, lens, emb, w3, b3, w4, b4, w5, b5)
    return out


# revision 15
# speedup vs baseline: 1.0505x; 1.0505x over previous
"""CharEmbeddingCNN Trainium2 kernel.

Reference computation (per word of L=20 chars):
    xe = emb[x]                       # [L, 256] -> treated as [256, L]
    y_k = conv1d_valid(xe, w_k) + b_k # k in (3,4,5), 256 -> 256 channels
    out = relu(max over all (k, t) of y_k[:, t]) * (len != 0)

Strategy (data-parallel over 8 NeuronCores, 1024 words each):
  - Host packs weights as bf16 lhsT tiles [i, (k,dk), o] and emb as a bf16
    DRAM table.
  - On device, `dma_gather(transpose=True)` gathers the embedding rows for
    a block of words directly into [128 part = ch%128, ch//128, pos] layout.
  - Conv = PSUM-accumulated matmuls over (i_chunk, dk) with shifted access
    patterns into the gathered block; word-groups of 28 give matmul free
    dims of 504/476/448 (= one PSUM bank).
  - Segment max over t via strided DVE reduce_max into per-k accumulators,
    then bias add + max-combine across k, PE transpose to [word, ch], and
    a fused relu*mask on ScalarE on the way out.
"""

import numpy as np
import ml_dtypes
from contextlib import ExitStack

import concourse.bass as bass
import concourse.bacc as bacc
import concourse.tile as tile
from concourse import masks, mybir
from concourse.bass_utils import run_bass_kernel_spmd

F32 = mybir.dt.float32
BF16 = mybir.dt.bfloat16
I16 = mybir.dt.int16

B, S, L = 64, 128, 20
EMB = 256
KS = (3, 4, 5)
NCORES = 8
W = (B * S) // NCORES          # words per core (1024)
GROUP = 24                     # words per matmul group (24*18=432 <= 512)
# Gather block: num_idxs = BLOCK*20 must be %128 == 0 (so BLOCK % 32 == 0)
# and <= ~2032 (SWDGE rx descriptor ring: num_idxs/8 + 2 <= 256 descs/lane).
BLOCK = 96
NKDK = sum(KS)                 # 12 packed (k, dk) weight slices


def _plan(words):
    """Split `words` into gather blocks (multiples of 32 words so that
    num_idxs % 128 == 0) and each block into matmul groups of <= GROUP.
    The first blocks are small so the matmul stream starts early."""
    blocks = []
    done = 0
    for bw0 in (32, 64):
        if words - done >= 2 * bw0:
            blocks.append(bw0)
            done += bw0
    while done < words:
        bw = min(BLOCK, words - done)
        assert (bw * L) % 128 == 0, (words, bw)
        blocks.append(bw)
        done += bw
    out = []
    done = 0
    for bw in blocks:
        groups = []
        g = 0
        while g < bw:
            gw = min(GROUP, bw - g)
            groups.append((g, gw))
            g += gw
        out.append((done, bw, groups))
        done += bw
    return out


def _kdk_off(ki, dk):
    return sum(KS[:ki]) + dk


def build_bass(words=W):
    """Build the per-core Bass module. Returns (nc, input names)."""
    assert words % 128 == 0
    nwb = words // 128          # output word-blocks of 128 (transpose granularity)
    blocks = _plan(words)

    nc = bacc.Bacc(
        "TRN2",
        target_bir_lowering=False,
        debug=False,
        enable_asserts=False,
        num_swdge_queues=4,
    )

    xi_cols = words * L // 16
    xi_d = nc.dram_tensor("xi", [128, xi_cols], I16, kind="ExternalInput").ap()
    emb_d = nc.dram_tensor("emb", [EMB, EMB], BF16, kind="ExternalInput").ap()
    wpk_d = nc.dram_tensor("wpk", [EMB, NKDK, EMB], BF16, kind="ExternalInput").ap()
    bias_d = nc.dram_tensor("bias", [128, 6], F32, kind="ExternalInput").ap()
    mask_d = nc.dram_tensor("maskp", [128, nwb], F32, kind="ExternalInput").ap()
    out_d = nc.dram_tensor("out", [words, EMB], F32, kind="ExternalOutput").ap()

    with tile.TileContext(nc) as tc, ExitStack() as ctx:
        const_pool = ctx.enter_context(tc.tile_pool(name="const", bufs=1))
        xe_pool = ctx.enter_context(tc.tile_pool(name="xe", bufs=2))
        psum_pool = ctx.enter_context(tc.tile_pool(name="ps", bufs=2, space="PSUM"))
        psum_t_pool = ctx.enter_context(tc.tile_pool(name="pst", bufs=2, space="PSUM"))
        m_pool = ctx.enter_context(tc.tile_pool(name="m", bufs=1))
        tmp_pool = ctx.enter_context(tc.tile_pool(name="tmp", bufs=2))
        out_pool = ctx.enter_context(tc.tile_pool(name="outp", bufs=3))

        ident = const_pool.tile([128, 128], F32)
        masks.make_identity(nc, ident[:])

        xi_t = const_pool.tile([128, xi_cols], I16)
        nc.sync.dma_start(xi_t[:], xi_d[:])
        wt = []
        for ic in range(2):
            t = const_pool.tile([128, NKDK, EMB], BF16, tag=f"wt{ic}")
            nc.sync.dma_start(t[:], wpk_d[ic * 128:(ic + 1) * 128, :, :])
            wt.append(t)
        bias_t = const_pool.tile([128, 6], F32)
        nc.sync.dma_start(bias_t[:], bias_d[:])
        mask_t = const_pool.tile([128, nwb], F32)
        nc.sync.dma_start(mask_t[:], mask_d[:])

        # per-(k, o_chunk) running max accumulators over all words
        M = {}
        for ki in range(3):
            for oc in range(2):
                M[(ki, oc)] = m_pool.tile(
                    [128, words], F32, tag=f"m{ki}{oc}", name=f"m{ki}{oc}")
        # combined (biased, maxed-over-k) result, per o_chunk
        C = [m_pool.tile([128, words], F32, tag=f"c{oc}", name=f"c{oc}")
             for oc in range(2)]

        # PE warm-up: dummy matmuls keep TensorE busy (HAM -> K=8/8) while
        # the first gather's descriptor generation runs on GpSimd.
        scratch = const_pool.tile([128, 512], BF16)
        nc.vector.memset(scratch[:], 0.0)
        warm = psum_pool.tile([128, 512], F32, tag="ps0")
        for _ in range(24):
            nc.tensor.matmul(warm[:], scratch[:, :128], scratch[:], start=True,
                             stop=True)

        def emit_output(wb):
            for oc in range(2):
                pst = psum_t_pool.tile([128, 128], F32, tag="pst", name="pst")
                nc.tensor.transpose(
                    pst[:], C[oc][:, wb * 128:(wb + 1) * 128], ident[:])
                ot = out_pool.tile([128, 128], F32, tag="ot", name="ot")
                nc.scalar.activation(
                    ot[:], pst[:], mybir.ActivationFunctionType.Relu,
                    scale=mask_t[:, wb:wb + 1],
                )
                nc.sync.dma_start(
                    out_d[wb * 128:(wb + 1) * 128, oc * 128:(oc + 1) * 128],
                    ot[:],
                )

        wb_done = 0
        for bi, (w0, bw, groups) in enumerate(blocks):
            nidx = bw * L
            xe = xe_pool.tile([128, 2, nidx], BF16, tag="xe")
            nc.gpsimd.dma_gather(
                xe[:], emb_d[:], xi_t[:, w0 * L // 16: (w0 + bw) * L // 16],
                nidx, nidx, EMB, transpose=True, single_packet=False,
                queue_num=bi % 4,
            )
            # [128, ch_chunk, word, t] views of the gathered block
            xv = [xe[:, ic, :].rearrange("p (w t) -> p w t", t=L) for ic in range(2)]
            for (g0, gw) in groups:
                for oc in range(2):
                    for ki, k in enumerate(KS):
                        lk = L - k + 1
                        ps = psum_pool.tile([128, gw, lk], F32, tag=f"ps{ki}")
                        n = 2 * k
                        i = 0
                        for ic in range(2):
                            for dk in range(k):
                                nc.tensor.matmul(
                                    ps[:],
                                    wt[ic][:, _kdk_off(ki, dk),
                                           oc * 128:(oc + 1) * 128],
                                    xv[ic][:, g0:g0 + gw, dk:dk + lk],
                                    start=(i == 0),
                                    stop=(i == n - 1),
                                )
                                i += 1
                        c0 = w0 + g0
                        nc.vector.reduce_max(
                            M[(ki, oc)][:, c0:c0 + gw], ps[:],
                            axis=mybir.AxisListType.X,
                        )
            # fold this block's columns into C (bias + max over k)
            sl = slice(w0, w0 + bw)
            for oc in range(2):
                t4 = tmp_pool.tile([128, bw], F32, tag="t4")
                nc.vector.tensor_scalar_add(
                    C[oc][:, sl], M[(0, oc)][:, sl], bias_t[:, 3 * oc:3 * oc + 1])
                nc.vector.tensor_scalar_add(
                    t4[:], M[(1, oc)][:, sl], bias_t[:, 3 * oc + 1:3 * oc + 2])
                nc.vector.tensor_max(C[oc][:, sl], C[oc][:, sl], t4[:])
                nc.vector.tensor_scalar_add(
                    t4[:], M[(2, oc)][:, sl], bias_t[:, 3 * oc + 2:3 * oc + 3])
                nc.vector.tensor_max(C[oc][:, sl], C[oc][:, sl], t4[:])
            # emit finished 128-word output blocks
            while (wb_done + 1) * 128 <= w0 + bw:
                emit_output(wb_done)
                wb_done += 1
        while wb_done < nwb:
            emit_output(wb_done)
            wb_done += 1
    nc.compile()
    return nc


def prep_shared(emb, w3, w4, w5, b3, b4, b5):
    emb_bf = np.ascontiguousarray(emb.astype(ml_dtypes.bfloat16))
    wpk = np.empty((EMB, NKDK, EMB), dtype=ml_dtypes.bfloat16)
    for ki, w in enumerate((w3, w4, w5)):
        k = KS[ki]
        for dk in range(k):
            # wpk[i, off, o] = w[o, i, dk]
            wpk[:, _kdk_off(ki, dk), :] = w[:, :, dk].T.astype(ml_dtypes.bfloat16)
    bias = np.empty((128, 6), dtype=np.float32)
    for oc in range(2):
        for ki, b in enumerate((b3, b4, b5)):
            bias[:, 3 * oc + ki] = b[oc * 128:(oc + 1) * 128]
    return emb_bf, wpk, bias


def prep_core(xf, lensf, words=W):
    """Per-core index + mask packing. xf: [words, L] int32, lensf: [words]."""
    xi = xf.reshape(-1).astype(np.int16)               # words * L
    # dma_gather index layout: idx j -> partition j % 16, column j // 16,
    # replicated across the 8 gpsimd cores (16-partition stripes).
    cols = xi.reshape(-1, 16).T                        # [16, words*L/16]
    xi_t = np.ascontiguousarray(np.tile(cols, (8, 1)))  # [128, cols]
    nwb = words // 128
    maskp = (lensf.reshape(nwb, 128).T != 0).astype(np.float32)
    maskp = np.ascontiguousarray(maskp)                # [128, nwb]
    return xi_t, maskp


_CACHE = {}


def _get_nc(words=W):
    if words not in _CACHE:
        _CACHE[words] = build_bass(words)
    return _CACHE[words]


def run(x, lens, emb, w3, b3, w4, b4, w5, b5, trace=False, **spmd_kwargs):
    x = np.asarray(x)
    lens = np.asarray(lens)
    emb = np.asarray(emb, dtype=np.float32)
    nc = _get_nc()
    emb_bf, wpk, bias = prep_shared(
        np.asarray(emb), np.asarray(w3), np.asarray(w4), np.asarray(w5),
        np.asarray(b3), np.asarray(b4), np.asarray(b5))
    xf = x.reshape(B * S, L)
    lensf = lens.reshape(B * S)
    in_maps = []
    for c in range(NCORES):
        sl = slice(c * W, (c + 1) * W)
        xi_t, maskp = prep_core(xf[sl], lensf[sl])
        in_maps.append({
            "xi": xi_t, "emb": emb_bf, "wpk": wpk, "bias": bias, "maskp": maskp,
        })
    res = run_bass_kernel_spmd(
        nc, in_maps, core_ids=list(range(NCORES)), trace=trace, **spmd_kwargs)
    out = np.concatenate([r["out"] for r in res.results], axis=0)
    return np.ascontiguousarray(out.reshape(B, S, EMB).astype(np.float32)), res


def kernel(x, lens, emb, w3, b3, w4, b4, w5, b5, **unused):
    out, _ = run(x, lens, emb, w3, b3, w4, b4, w5, b5)
    return out
